# revision 12
# baseline (speedup 1.0000x reference)
"""CRF mean-log-likelihood kernel for Trainium2, 8 NeuronCores, data-parallel.

Problem: B=256, M=1024, D=128, N=26.
  e = X @ W.T ; prob-space forward scan f <- exp(e_i) * (exp(T)^T f);
  result = mean_j [ sum_p e[p, y_p] + sum_p T[y_p, y_p+1] - logZ_j ].

Sharding: batch across 8 cores (32 seqs/core). Device computes, per core:
  - e via PE (X^T-block stationary, W^T moving) in natural [pos, label] layout
  - emission-gather sums via GPSIMD indirect_copy + ACT Ln + DVE masked reduce
  - logZ via a chunked rank-1 two-pass scan (K=8 positions/chunk):
      pass1: w_c = P_c @ 1 for all chunks in parallel (8 serial steps)
      pass2: v_c = P_c @ w_{c-1}; logZ telescopes into log-ratios of column sums
    4 chunk-bands folded onto partitions (blockdiag exp(T) lhsT), so each scan
    step is one [128x128]x[128,cols] matmul + one DVE multiply by exp(e).
Host finishes with tiny assembly: logs of per-chunk sums, transition term,
and the mean. Heavy data (X) is touched only on device.

Position coords per seq: p = 512*Wseq + 128*fb + 8*sub + k
  (global W-block w = 2*j_core + Wseq; band fb = partition band of the chunk;
   chunk id c = 64*Wseq + 16*fb + sub; within-chunk step k).
"""
import sys
sys.path.insert(0, '/opt/trn_rl_repo')
import numpy as np
import ml_dtypes

bf16 = ml_dtypes.bfloat16

B, M, D, N = 256, 1024, 128, 26
NCORES = 8
S = B // NCORES          # 32 seqs per core
NWc = 2 * S              # 64 W-blocks (512 positions) per core
NGRP = 4                 # seq groups per core (pipelining granularity)
SG = S // NGRP           # 8 seqs per group
NWg = 2 * SG             # 16 W-blocks per group
K = 8                    # chunk length
NSUB = 16                # sub-chunks per (W, band)
COLS_G = NWg * NSUB      # 256 state columns per group
IMAX = 128               # padded picks per (label-group, residue)
GATH_G = 16 * IMAX       # 2048 gather slots per seq-group

_cache = {}


def _host_consts(W, T):
    Wt = np.ascontiguousarray(W.T).astype(bf16)              # [128, 26]
    Mt = np.exp(T.astype(np.float64))
    M4 = np.zeros((128, 128), np.float32)
    for g in range(4):
        M4[32 * g:32 * g + N, 32 * g:32 * g + N] = Mt
    P_up = np.zeros((128, 128), np.float32)
    for g in range(3):
        P_up[32 * g:32 * g + N, 32 * (g + 1):32 * (g + 1) + N] = np.eye(N)
    P_wrap = np.zeros((128, 128), np.float32)
    P_wrap[96:96 + N, 0:N] = np.eye(N)
    ones4 = np.zeros((128, 4), np.float32)
    for g in range(4):
        ones4[32 * g:32 * g + N, g] = 1.0
    rcs = (1.0 / Mt.sum(axis=0)).astype(np.float32)          # [26]
    rcs_col = np.zeros((N, 1), np.float32); rcs_col[:, 0] = rcs
    Wt128 = np.zeros((128, 128), np.float32)                 # band-3 stationary
    Wt128[:, 96:96 + N] = W.T
    return dict(Wt=Wt, Wt128=Wt128.astype(bf16), M4=M4.astype(bf16),
                P_up=P_up.astype(bf16), P_wrap=P_wrap.astype(bf16),
                ones4=ones4.astype(bf16), rcs_col=rcs_col)


def _gather_idx(labels_core):
    """Gather picks from E4 [32*fb+a, w_local*128+pp] per seq-group tile.
    Per 16-partition label-group gg = 2*fb + (y>=16), per residue r = y%16,
    slot 16*i + r picks column P - 2048*g; keeper partition = 32*fb + y.
    Returns idx uint16 [128, NGRP*IMAX] (wrapped) and mask bf16 [128, NGRP*16*IMAX]."""
    y = labels_core.reshape(-1).astype(np.int64)
    P = np.arange(S * M)
    fb = (P // 128) % 4
    g = P // 8192
    col = (P // 512 - 16 * g) * 128 + P % 128
    gg = 2 * fb + (y >= 16)
    r = y % 16
    part = 32 * fb + y
    idx = np.zeros((128, NGRP * IMAX), np.uint16)
    mask = np.zeros((128, NGRP * 16 * IMAX), np.float32)
    counters = np.zeros((NGRP, 8, 16), np.int64)
    for p_ in range(S * M):
        gi, ggi, ri = g[p_], gg[p_], r[p_]
        i = counters[gi, ggi, ri]
        counters[gi, ggi, ri] += 1
        assert i < IMAX, "gather slot overflow; raise IMAX"
        idx[16 * ggi + ri, gi * IMAX + i] = col[p_]
        mask[part[p_], gi * 16 * IMAX + 16 * i + ri] = 1.0
    return idx, mask.astype(bf16)


def _build_kernel():
    import concourse.bass as bass
    import concourse.mybir as mybir
    import concourse.tile as tile
    from concourse.masks import make_identity

    fp32 = mybir.dt.float32
    bfl = mybir.dt.bfloat16

    nc = bass.Bass()
    X_in = nc.dram_tensor("X", [S * M, D], fp32, kind="ExternalInput")
    idx_in = nc.dram_tensor("gidx", [128, NGRP * IMAX], mybir.dt.uint16, kind="ExternalInput")
    Wt_in = nc.dram_tensor("Wt", [128, N], bfl, kind="ExternalInput")
    Wt128_in = nc.dram_tensor("Wt128", [128, 128], bfl, kind="ExternalInput")
    M4_in = nc.dram_tensor("M4", [128, 128], bfl, kind="ExternalInput")
    Pup_in = nc.dram_tensor("P_up", [128, 128], bfl, kind="ExternalInput")
    Pwr_in = nc.dram_tensor("P_wrap", [128, 128], bfl, kind="ExternalInput")
    ones4_in = nc.dram_tensor("ones4", [128, 4], bfl, kind="ExternalInput")
    rcs_in = nc.dram_tensor("rcs_col", [N, 1], fp32, kind="ExternalInput")
    mask_in = nc.dram_tensor("mask", [128, NGRP * GATH_G], bfl, kind="ExternalInput")

    sw_out = nc.dram_tensor("s_w", [4, NWc * NSUB], fp32, kind="ExternalOutput")
    sv_out = nc.dram_tensor("s_v", [4, NWc * NSUB], fp32, kind="ExternalOutput")
    gs_out = nc.dram_tensor("gsum", [128, 2 * NGRP], fp32, kind="ExternalOutput")

    with tile.TileContext(nc) as tc:
        with tc.tile_pool(name="const", bufs=1) as cpool, \
             tc.tile_pool(name="xn", bufs=2) as xn_pool, \
             tc.tile_pool(name="xt", bufs=2) as xt_pool, \
             tc.tile_pool(name="u4p", bufs=2) as u4_pool, \
             tc.tile_pool(name="scr", bufs=2) as scr_pool, \
             tc.tile_pool(name="st", bufs=2) as st_pool, \
             tc.tile_pool(name="acc", bufs=1) as acc_pool, \
             tc.tile_pool(name="eps", bufs=2, space="PSUM") as eps_pool, \
             tc.tile_pool(name="sps", bufs=2, space="PSUM") as sps_pool, \
             tc.tile_pool(name="mps", bufs=1, space="PSUM") as mps_pool:

            Wt_sb = cpool.tile([128, N], bfl)
            nc.sync.dma_start(Wt_sb[:], Wt_in[:])
            Wt128_sb = cpool.tile([128, 128], bfl)
            nc.sync.dma_start(Wt128_sb[:], Wt128_in[:])
            M4_sb = cpool.tile([128, 128], bfl)
            nc.sync.dma_start(M4_sb[:], M4_in[:])
            Pup_sb = cpool.tile([128, 128], bfl)
            nc.sync.dma_start(Pup_sb[:], Pup_in[:])
            Pwr_sb = cpool.tile([128, 128], bfl)
            nc.sync.dma_start(Pwr_sb[:], Pwr_in[:])
            ones4_sb = cpool.tile([128, 4], bfl)
            nc.sync.dma_start(ones4_sb[:], ones4_in[:])
            rcs_sb = cpool.tile([N, 1], fp32)
            nc.sync.dma_start(rcs_sb[:], rcs_in[:])
            mask_sb = cpool.tile([128, NGRP * GATH_G], bfl)
            nc.sync.dma_start(mask_sb[:], mask_in[:])
            idx_sb = cpool.tile([128, NGRP * IMAX], mybir.dt.uint16)
            nc.sync.dma_start(idx_sb[:], idx_in[:])

            # persistent per-core buffers
            s_w = acc_pool.tile([4, NWc * NSUB], fp32)
            s_v = acc_pool.tile([4, NWc * NSUB], fp32)
            gsum = acc_pool.tile([128, 2 * NGRP], fp32)

            X_v = X_in[:].rearrange("(b p) d -> b p d", p=128)  # [256, 128, 128]

            for g in range(NGRP):
                # ---- load + cast X for this seq group: [128, 64 blk, 128] bf16
                xn = xn_pool.tile([128, NWg * 4, 128], bfl)
                nc.gpsimd.dma_start(
                    xn[:], X_v[g * 64:(g + 1) * 64].rearrange("b p d -> p b d"))
                xt = xt_pool.tile([128, NWg * 4, 128], bfl)
                nc.sync.dma_start_transpose(
                    xt[:], xn[:].rearrange("p b d -> p (b d)"))
                xt4 = xt[:].rearrange("d (w f) p -> d w f p", f=4)

                # ---- e-matmul straight into band-folded layout:
                #      psum[32*fb + a, (w4, pp)] = e[pos(w, fb, pp), a]
                # band 3 via full-width zero-padded stationary (writes all 128
                # rows incl. zero pads), bands 0-2 overwrite their 26-row slices.
                u4g = u4_pool.tile([128, NWg * 128], bfl, tag="u4")
                e4g = u4_pool.tile([128, NWg * 128], bfl, tag="e4")
                for q in range(4):                   # 4 chunks of 4 W-blocks
                    ep = eps_pool.tile([128, 512], fp32)
                    nc.tensor.matmul(ep[:], Wt128_sb[:],
                                     xt4[:, 4 * q:4 * q + 4, 3, :],
                                     start=True, stop=True)
                    for fb in range(3):
                        nc.tensor.matmul(ep[32 * fb:32 * fb + N, :], Wt_sb[:],
                                         xt4[:, 4 * q:4 * q + 4, fb, :],
                                         start=True, stop=True,
                                         skip_group_check=True)
                    nc.scalar.copy(e4g[:, q * 512:(q + 1) * 512], ep[:])
                    nc.scalar.activation(u4g[:, q * 512:(q + 1) * 512], ep[:],
                                         mybir.ActivationFunctionType.Exp)

                # ---- emission gather from raw e (per-seq-group tile)
                H = IMAX // 2
                for h in range(2):
                    picked = scr_pool.tile([128, GATH_G // 2], bfl)
                    nc.gpsimd.indirect_copy(
                        picked[:], e4g[:],
                        idx_sb[:, g * IMAX + h * H:g * IMAX + (h + 1) * H],
                        i_know_ap_gather_is_preferred=True)
                    masked = scr_pool.tile([128, GATH_G // 2], fp32)
                    nc.vector.tensor_mul(
                        masked[:], picked[:],
                        mask_sb[:, g * GATH_G + h * 16 * H:
                                g * GATH_G + (h + 1) * 16 * H])
                    nc.vector.tensor_reduce(gsum[:, 2 * g + h:2 * g + h + 1],
                                            masked[:], mybir.AxisListType.X,
                                            mybir.AluOpType.add)

                # ---- u0 fix: position 0 of each seq (w_local=2jj, fb=0, pp=0)
                u0 = u4g[:].rearrange("q (j c) -> q j c", c=256)
                nc.vector.tensor_scalar_mul(u0[0:N, :, 0], u0[0:N, :, 0],
                                            rcs_sb[:])

                # ---- two-pass chunked scan for this group
                u4s = u4g[:].rearrange("q (w s k) -> q w s k", s=NSUB, k=K)

                def scan_pass(st_init):
                    st = st_init
                    for k in range(K):
                        mp = sps_pool.tile([128, COLS_G], fp32)
                        nc.tensor.matmul(mp[:], M4_sb[:], st[:],
                                         start=True, stop=True)
                        stn = st_pool.tile([128, COLS_G], bfl)
                        uop = u4s[:, :, :, k].rearrange("q w s -> q (w s)")
                        nc.vector.tensor_mul(stn[:], mp[:], uop)
                        st = stn
                    return st

                st1 = st_pool.tile([128, COLS_G], bfl)
                nc.gpsimd.memset(st1[:], 0.0)
                for fb in range(4):
                    nc.gpsimd.memset(st1[32 * fb:32 * fb + N, :], 1.0)
                wfin = scan_pass(st1)

                op = mps_pool.tile([4, COLS_G], fp32, tag="ones_out")
                nc.tensor.matmul(op[:], ones4_sb[:], wfin[:], start=True, stop=True)
                nc.vector.tensor_copy(s_w[:, g * COLS_G:(g + 1) * COLS_G], op[:])

                # pass-2 init: shift state by one chunk
                st2 = st_pool.tile([128, COLS_G], bfl)
                wv = wfin[:].rearrange("q (w s) -> q w s", s=NSUB)
                sv2 = st2[:].rearrange("q (w s) -> q w s", s=NSUB)
                nc.vector.tensor_copy(sv2[:, :, 1:NSUB], wv[:, :, 0:NSUB - 1])
                rp = mps_pool.tile([128, NWg], fp32, tag="rot_out")
                wv2 = wfin[:].rearrange("q (w2 e s) -> q w2 e s", e=2, s=NSUB)
                rp2 = rp[:].rearrange("q (w2 e) -> q w2 e", e=2)
                nc.tensor.matmul(rp[:], Pup_sb[:], wv[:, :, NSUB - 1],
                                 start=True, stop=False)
                nc.tensor.matmul(rp2[:, :, 1], Pwr_sb[:],
                                 wv2[:, :, 0, NSUB - 1],
                                 start=False, stop=True, skip_group_check=True)
                nc.vector.tensor_copy(sv2[:, :, 0], rp[:])
                sv2r = st2[:].rearrange("q (w2 e s) -> q w2 e s", e=2, s=NSUB)
                nc.gpsimd.memset(sv2r[0:N, :, 0, 0], 1.0)    # dummy init, c=0 chunks
                vfin = scan_pass(st2)

                ov = mps_pool.tile([4, COLS_G], fp32, tag="ones_out")
                nc.tensor.matmul(ov[:], ones4_sb[:], vfin[:], start=True, stop=True)
                nc.vector.tensor_copy(s_v[:, g * COLS_G:(g + 1) * COLS_G], ov[:])

            nc.sync.dma_start(sw_out[:], s_w[:])
            nc.sync.dma_start(sv_out[:], s_v[:])
            nc.sync.dma_start(gs_out[:], gsum[:])

    _split_multi_waits(nc)
    return nc


def _split_multi_waits(nc, max_waits=1):
    """This walrus build allows one sem-wait per instruction; split extras
    into standalone EventSemaphore instructions on the same engine."""
    import concourse.mybir as mybir
    import bass_rust
    total = 0
    for fn in nc.m.functions:
        for bb in fn.blocks:
            insts = list(bb.instructions)
            out = []
            for ins in insts:
                si = ins.sync_info
                if si is not None and len(si.on_wait) > max_waits:
                    waits = list(si.on_wait)
                    for j, w in enumerate(waits[max_waits:]):
                        ev = mybir.InstEventSemaphore(
                            name=f'{ins.name}-xw{j}', ins=[], outs=[])
                        ev.sync_info = bass_rust.SyncInfo(on_wait=[w], on_update=[])
                        ev.engine = ins.engine
                        out.append(ev)
                        total += 1
                    ins.sync_info = bass_rust.SyncInfo(
                        on_wait=waits[:max_waits], on_update=list(si.on_update))
                out.append(ins)
            bb.instructions.clear()
            bb.instructions.extend(out)
    return total


def kernel(X, labels, W, T, _trace=False):
    from concourse.bass_utils import run_bass_kernel_spmd

    if 'nc' not in _cache:
        _cache['nc'] = _build_kernel()
    nc = _cache['nc']

    consts = _host_consts(W, T)
    in_maps = []
    for c in range(NCORES):
        Xc = np.ascontiguousarray(X[S * c:S * (c + 1)]).reshape(S * M, D)
        gidx, gmask = _gather_idx(labels[S * c:S * (c + 1)])
        m = {"X": Xc, "gidx": gidx, "mask": gmask}
        m.update(consts)
        in_maps.append(m)

    out = run_bass_kernel_spmd(nc, in_maps, core_ids=list(range(NCORES)),
                               trace=_trace)
    results = out.results

    total = 0.0
    for c in range(NCORES):
        r = results[c]
        s_w = np.asarray(r["s_w"], np.float64)       # [4, 1024], col = w*16+sub
        s_v = np.asarray(r["s_v"], np.float64)
        gsum = float(np.asarray(r["gsum"], np.float64).sum())
        ls_w, ls_v = np.log(s_w), np.log(s_v)
        total_v, total_w = ls_v.sum(), ls_w.sum()
        j = np.arange(S)
        corr_v = ls_v[0, 32 * j].sum()               # c=0 dummy cols (band0, w=2j, sub0)
        corr_w0 = ls_w[0, 32 * j].sum()
        corr_w127 = ls_w[3, (2 * j + 1) * 16 + 15].sum()   # c=127 (band3, w odd, sub15)
        logZ_sum = corr_w0 + (total_v - corr_v) - (total_w - corr_w127)
        yc = labels[S * c:S * (c + 1)]
        trans = float(T[yc[:, :-1], yc[:, 1:]].sum(dtype=np.float64))
        total += gsum + trans - logZ_sum

    res = np.float32(total / B)
    if _trace:
        return res, out
    return res
